# revision 22
# baseline (speedup 1.0000x reference)
"""Bahdanau-style cosine attention kernel for Trainium2 (8 NeuronCores).

reference math (fp32):
    q = squeeze(query)              # [H]
    dots = keys @ q                 # [S]
    cos = dots / (|q| * |keys_i|)   # [S]
    context = sum_i cos_i * keys_i  # [H]

Sharding: keys split along S across 8 cores (4096 rows each); query is
normalized by |q| on the host, cast to fp16 and broadcast to 128
partitions; keys are cast to fp16 on the host (rel err ~2e-4 vs the 2e-2
gate) so each core streams 8 MiB instead of 16 MiB.

Engine model (measured): accumulate passes only exist on DVE
(scalar_tensor_tensor, 1305 ns/tile incl. accumulator read — no 2x uop)
and ACT (Square+accum, 1125 ns/tile). dots can only run on DVE (per-
column multiplier), so: DVE = 32 dot passes, ACT = 32 square passes.
PE does all context matmuls (fp16, 2 PSUM bank pairs: the last small
group accumulates separately so the first 30 tiles' context can drain
to HBM while the DVE finishes the tail).
"""

import os
import sys

import numpy as np

for _p in ("/opt/trn_rl_repo",):
    if os.path.isdir(_p) and _p not in sys.path:
        sys.path.append(_p)

P = 128          # SBUF partitions
H = 1024         # feature dim
S_FULL = 32768   # full sequence
N_CORES = 8
S = S_FULL // N_CORES   # rows per core = 4096
T = S // P              # row-tiles per core = 32

# DMA chunks in tiles (1 tile = 128 rows = 256 KiB fp16). Small first
# chunks let compute start early; small last chunks shrink the tail.
CHUNKS = [2, 2, 4, 4, 4, 4, 4, 4, 2, 2]
assert sum(CHUNKS) == T
GROUPS = []
_t0 = 0
for _ct in CHUNKS:
    GROUPS.append((_t0, _t0 + _ct))
    _t0 += _ct
# groups whose ctx goes to PSUM bank pair B (drained at the very end);
# everything else accumulates in pair A which stops + drains early
B_GROUPS = frozenset({len(GROUPS) - 1})
PE_WARMUP_MMS = 6

_NC_CACHE = {}


def _build_nc():
    import concourse.bacc as bacc
    import concourse.tile as tile
    from concourse import mybir

    f32 = mybir.dt.float32
    f16 = mybir.dt.float16
    AF = mybir.ActivationFunctionType
    OP = mybir.AluOpType
    nc = bacc.Bacc("TRN2", target_bir_lowering=False, debug=False)

    keys_d = nc.dram_tensor("keys", [S, H], f16, kind="ExternalInput").ap()
    qb_d = nc.dram_tensor("qb", [P, H], f16, kind="ExternalInput").ap()
    ctx_d = nc.dram_tensor("ctx", [2, H], f32, kind="ExternalOutput").ap()

    with tile.TileContext(nc) as tc:
        with (
            tc.tile_pool(name="main", bufs=1) as pool,
            tc.tile_pool(name="psum", bufs=1, space="PSUM") as pp,
        ):
            qb = pool.tile([P, H], f16, name="qb_sb")
            nc.scalar.dma_start(qb[:], qb_d[:])

            # keys[p*T + t, c] -> sbuf[p, t, c]: per-partition DRAM runs
            # are contiguous fp16 -> few, large DMA descriptors
            keys_r = keys_d.rearrange("(p t) c -> p t c", p=P)
            kcs = []
            t0 = 0
            for j, ct in enumerate(CHUNKS):
                kc = pool.tile([P, ct * H], f16, name=f"kc{j}", tag=f"kc{j}")
                nc.sync.dma_start(kc[:], keys_r[:, t0 : t0 + ct, :])
                kcs.append((kc, t0, ct))
                t0 += ct

            tile_of = {}
            for kc, t0, ct in kcs:
                for i in range(ct):
                    tile_of[t0 + i] = (kc, i)

            def ktile(t):
                kc, i = tile_of[t]
                return kc[:, i * H : (i + 1) * H]

            dots = pool.tile([P, T], f32, name="dots")
            nrm2 = pp.tile([P, T], f32, name="nrm2")
            knrm = pool.tile([P, T], f32, name="knrm")
            rkn = pool.tile([P, T], f32, name="rkn")
            cosb = pool.tile([P, T], f16, name="cosb")
            dvescr = pool.tile([P, H], f16, name="dvescr")
            actscr = pp.tile([P, H], f32, name="actscr")
            psA0 = pp.tile([1, 512], f32, name="psA0")
            psA1 = pp.tile([1, 512], f32, name="psA1")
            psB0 = pp.tile([1, 512], f32, name="psB0")
            psB1 = pp.tile([1, 512], f32, name="psB1")

            # Warm the PE clock (HAM) during the DMA prologue; pair-B
            # banks are overwritten later by their first start=True MM.
            for _ in range(PE_WARMUP_MMS):
                nc.tensor.matmul(psB0[:], qb[:, 0:1], qb[:, 0:512],
                                 start=True, stop=True)

            firstA = {"v": True}
            firstB = {"v": True}

            def emit_group(gi, g0, g1):
                for t in range(g0, g1):
                    nc.vector.scalar_tensor_tensor(
                        out=dvescr[:], in0=ktile(t), scalar=1.0, in1=qb[:],
                        op0=OP.mult, op1=OP.mult,
                        accum_out=dots[:, t : t + 1],
                    )
                    nc.scalar.activation(
                        actscr[:], ktile(t), AF.Square,
                        accum_out=nrm2[:, t : t + 1],
                    )
                cols = slice(g0, g1)
                with tc.high_priority(offset=40):
                    nc.scalar.activation(knrm[:, cols], nrm2[:, cols],
                                         AF.Sqrt)
                    nc.vector.reciprocal(rkn[:, cols], knrm[:, cols])
                    nc.vector.tensor_mul(cosb[:, cols], dots[:, cols],
                                         rkn[:, cols])
                in_b = gi in B_GROUPS
                p0, p1 = (psB0, psB1) if in_b else (psA0, psA1)
                first = firstB if in_b else firstA
                lastA = (not in_b) and gi == max(
                    i for i in range(len(GROUPS)) if i not in B_GROUPS)
                lastB = in_b and gi == max(B_GROUPS)
                for t in range(g0, g1):
                    kt = ktile(t)
                    st = first["v"]
                    first["v"] = False
                    stop = (lastA or lastB) and t == g1 - 1
                    nc.tensor.matmul(p0[:], cosb[:, t : t + 1],
                                     kt[:, 0:512], start=st, stop=stop)
                    nc.tensor.matmul(p1[:], cosb[:, t : t + 1],
                                     kt[:, 512:1024], start=st, stop=stop)

            for gi, (g0, g1) in enumerate(GROUPS):
                emit_group(gi, g0, g1)

            # pair A drains (and its half of the output ships) while the
            # DVE still grinds the tail group's dots
            ctxA = pool.tile([1, H], f32, name="ctxA")
            ctxB = pool.tile([1, H], f32, name="ctxB")
            nc.scalar.copy(ctxA[:, 0:512], psA0[:])
            nc.vector.tensor_copy(ctxA[:, 512:1024], psA1[:])
            nc.scalar.dma_start(ctx_d[0:1, :], ctxA[:])
            nc.scalar.copy(ctxB[:, 0:512], psB0[:])
            nc.vector.tensor_copy(ctxB[:, 512:1024], psB1[:])
            nc.scalar.dma_start(ctx_d[1:2, :], ctxB[:])

    nc.compile()
    return nc


def _get_nc():
    if "nc" not in _NC_CACHE:
        _NC_CACHE["nc"] = _build_nc()
    return _NC_CACHE["nc"]


def prepare_in_maps(query: np.ndarray, keys: np.ndarray) -> list[dict]:
    query = np.asarray(query, dtype=np.float32)
    keys = np.asarray(keys, dtype=np.float32)
    assert query.shape == (1, H) and keys.shape == (S_FULL, H)

    q = query.reshape(H).astype(np.float64)
    qn = (q / np.linalg.norm(q)).astype(np.float16)
    qb = np.ascontiguousarray(np.broadcast_to(qn[None, :], (P, H)))

    keys16 = np.ascontiguousarray(keys.astype(np.float16))
    shards = keys16.reshape(N_CORES, S, H)
    return [{"keys": shards[i], "qb": qb} for i in range(N_CORES)]


def combine_results(results: list[dict]) -> np.ndarray:
    partials = np.stack([results[i]["ctx"] for i in range(N_CORES)])
    out = partials.astype(np.float64).sum(axis=(0, 1)).astype(np.float32)
    return out[None, :]


def kernel(query: np.ndarray, keys: np.ndarray) -> np.ndarray:
    from concourse.bass_utils import run_bass_kernel_spmd

    in_maps = prepare_in_maps(query, keys)
    nc = _get_nc()
    res = run_bass_kernel_spmd(nc, in_maps, list(range(N_CORES)))
    return combine_results(res.results)


# revision 26
# speedup vs baseline: 1.0126x; 1.0126x over previous
"""Bahdanau-style cosine attention kernel for Trainium2 (8 NeuronCores).

reference math (fp32):
    q = squeeze(query)              # [H]
    dots = keys @ q                 # [S]
    cos = dots / (|q| * |keys_i|)   # [S]
    context = sum_i cos_i * keys_i  # [H]

Sharding: keys split along S across 8 cores (4096 rows each); query is
normalized by |q| on the host, cast to fp16 and broadcast to 128
partitions; keys are cast to fp16 on the host (rel err ~2e-4 vs the 2e-2
gate) so each core streams 8 MiB instead of 16 MiB.

Engine model (measured): accumulate passes only exist on DVE
(scalar_tensor_tensor, 1305 ns/tile incl. accumulator read — no 2x uop)
and ACT (Square+accum, 1125 ns/tile). dots can only run on DVE (per-
column multiplier), so: DVE = 32 dot passes, ACT = 32 square passes.
PE does all context matmuls (fp16, 2 PSUM bank pairs: the last small
group accumulates separately so the first 30 tiles' context can drain
to HBM while the DVE finishes the tail).
"""

import os
import sys

import numpy as np

for _p in ("/opt/trn_rl_repo",):
    if os.path.isdir(_p) and _p not in sys.path:
        sys.path.append(_p)

P = 128          # SBUF partitions
H = 1024         # feature dim
S_FULL = 32768   # full sequence
N_CORES = 8
S = S_FULL // N_CORES   # rows per core = 4096
T = S // P              # row-tiles per core = 32

# DMA chunks in tiles (1 tile = 128 rows = 256 KiB fp16). Small first
# chunks let compute start early; small last chunks shrink the tail.
CHUNKS = [2, 2, 4, 4, 4, 4, 4, 4, 2, 2]
assert sum(CHUNKS) == T
GROUPS = []
_t0 = 0
for _ct in CHUNKS:
    GROUPS.append((_t0, _t0 + _ct))
    _t0 += _ct
# groups whose ctx goes to PSUM bank pair B (drained at the very end);
# everything else accumulates in pair A which stops + drains early
B_GROUPS = frozenset({len(GROUPS) - 1})
# tiles whose square runs on DVE instead of ACT (same PSUM accumulator;
# DVE stt is 1x regardless, so PSUM accum costs nothing extra) — shifts
# ~1.1us/tile off the ACT queue, the measured long pole
DVE_SQ_TILES = frozenset({12})
PE_WARMUP_MMS = 6

_NC_CACHE = {}


def _build_nc():
    import concourse.bacc as bacc
    import concourse.tile as tile
    from concourse import mybir

    f32 = mybir.dt.float32
    f16 = mybir.dt.float16
    AF = mybir.ActivationFunctionType
    OP = mybir.AluOpType
    nc = bacc.Bacc("TRN2", target_bir_lowering=False, debug=False)

    keys_d = nc.dram_tensor("keys", [S, H], f16, kind="ExternalInput").ap()
    qb_d = nc.dram_tensor("qb", [P, H], f16, kind="ExternalInput").ap()
    ctx_d = nc.dram_tensor("ctx", [2, H], f32, kind="ExternalOutput").ap()

    with tile.TileContext(nc) as tc:
        with (
            tc.tile_pool(name="main", bufs=1) as pool,
            tc.tile_pool(name="psum", bufs=1, space="PSUM") as pp,
        ):
            qb = pool.tile([P, H], f16, name="qb_sb")
            nc.sync.dma_start(qb[:], qb_d[:])

            # touch Square+Sqrt during the DMA prologue so both ACT
            # table loads (1.3us each) overlap idle time instead of
            # parking mid-stream on the ACT queue
            tblscr = pool.tile([1, 2], f32, name="tblscr")
            nc.vector.memset(tblscr[:], 1.0)
            nc.scalar.activation(tblscr[:], tblscr[:], AF.Square)
            nc.scalar.activation(tblscr[:], tblscr[:], AF.Sqrt)

            # keys[p*T + t, c] -> sbuf[p, t, c]: per-partition DRAM runs
            # are contiguous fp16 -> few, large DMA descriptors
            keys_r = keys_d.rearrange("(p t) c -> p t c", p=P)
            kcs = []
            t0 = 0
            for j, ct in enumerate(CHUNKS):
                kc = pool.tile([P, ct * H], f16, name=f"kc{j}", tag=f"kc{j}")
                nc.sync.dma_start(kc[:], keys_r[:, t0 : t0 + ct, :])
                kcs.append((kc, t0, ct))
                t0 += ct

            tile_of = {}
            for kc, t0, ct in kcs:
                for i in range(ct):
                    tile_of[t0 + i] = (kc, i)

            def ktile(t):
                kc, i = tile_of[t]
                return kc[:, i * H : (i + 1) * H]

            dots = pool.tile([P, T], f32, name="dots")
            nrm2 = pp.tile([P, T], f32, name="nrm2")
            knrm = pool.tile([P, T], f32, name="knrm")
            rkn = pool.tile([P, T], f32, name="rkn")
            cosb = pool.tile([P, T], f16, name="cosb")
            dvescr = pool.tile([P, H], f16, name="dvescr")
            actscr = pp.tile([P, H], f32, name="actscr")
            psA0 = pp.tile([1, 512], f32, name="psA0")
            psA1 = pp.tile([1, 512], f32, name="psA1")
            psB0 = pp.tile([1, 512], f32, name="psB0")
            psB1 = pp.tile([1, 512], f32, name="psB1")

            # Warm the PE clock (HAM) during the DMA prologue; pair-B
            # banks are overwritten later by their first start=True MM.
            for _ in range(PE_WARMUP_MMS):
                nc.tensor.matmul(psB0[:], qb[:, 0:1], qb[:, 0:512],
                                 start=True, stop=True)

            firstA = {"v": True}
            firstB = {"v": True}

            def emit_group(gi, g0, g1):
                for t in range(g0, g1):
                    nc.vector.scalar_tensor_tensor(
                        out=dvescr[:], in0=ktile(t), scalar=1.0, in1=qb[:],
                        op0=OP.mult, op1=OP.mult,
                        accum_out=dots[:, t : t + 1],
                    )
                    if t in DVE_SQ_TILES:
                        nc.vector.scalar_tensor_tensor(
                            out=dvescr[:], in0=ktile(t), scalar=1.0,
                            in1=ktile(t), op0=OP.mult, op1=OP.mult,
                            accum_out=nrm2[:, t : t + 1],
                        )
                    else:
                        nc.scalar.activation(
                            actscr[:], ktile(t), AF.Square,
                            accum_out=nrm2[:, t : t + 1],
                        )
                cols = slice(g0, g1)
                with tc.high_priority(offset=40):
                    nc.scalar.activation(knrm[:, cols], nrm2[:, cols],
                                         AF.Sqrt)
                    nc.vector.reciprocal(rkn[:, cols], knrm[:, cols])
                    nc.vector.tensor_mul(cosb[:, cols], dots[:, cols],
                                         rkn[:, cols])
                in_b = gi in B_GROUPS
                p0, p1 = (psB0, psB1) if in_b else (psA0, psA1)
                first = firstB if in_b else firstA
                lastA = (not in_b) and gi == max(
                    i for i in range(len(GROUPS)) if i not in B_GROUPS)
                lastB = in_b and gi == max(B_GROUPS)
                for t in range(g0, g1):
                    kt = ktile(t)
                    st = first["v"]
                    first["v"] = False
                    stop = (lastA or lastB) and t == g1 - 1
                    nc.tensor.matmul(p0[:], cosb[:, t : t + 1],
                                     kt[:, 0:512], start=st, stop=stop)
                    nc.tensor.matmul(p1[:], cosb[:, t : t + 1],
                                     kt[:, 512:1024], start=st, stop=stop)

            for gi, (g0, g1) in enumerate(GROUPS):
                emit_group(gi, g0, g1)

            # pair A drains (and its half of the output ships) while the
            # DVE still grinds the tail group's dots
            ctxA = pool.tile([1, H], f32, name="ctxA")
            ctxB = pool.tile([1, H], f32, name="ctxB")
            nc.scalar.copy(ctxA[:, 0:512], psA0[:])
            nc.vector.tensor_copy(ctxA[:, 512:1024], psA1[:])
            nc.sync.dma_start(ctx_d[0:1, :], ctxA[:])
            nc.scalar.copy(ctxB[:, 0:512], psB0[:])
            nc.vector.tensor_copy(ctxB[:, 512:1024], psB1[:])
            nc.sync.dma_start(ctx_d[1:2, :], ctxB[:])

    nc.compile()
    return nc


def _get_nc():
    if "nc" not in _NC_CACHE:
        _NC_CACHE["nc"] = _build_nc()
    return _NC_CACHE["nc"]


def prepare_in_maps(query: np.ndarray, keys: np.ndarray) -> list[dict]:
    query = np.asarray(query, dtype=np.float32)
    keys = np.asarray(keys, dtype=np.float32)
    assert query.shape == (1, H) and keys.shape == (S_FULL, H)

    q = query.reshape(H).astype(np.float64)
    qn = (q / np.linalg.norm(q)).astype(np.float16)
    qb = np.ascontiguousarray(np.broadcast_to(qn[None, :], (P, H)))

    keys16 = np.ascontiguousarray(keys.astype(np.float16))
    shards = keys16.reshape(N_CORES, S, H)
    return [{"keys": shards[i], "qb": qb} for i in range(N_CORES)]


def combine_results(results: list[dict]) -> np.ndarray:
    partials = np.stack([results[i]["ctx"] for i in range(N_CORES)])
    out = partials.astype(np.float64).sum(axis=(0, 1)).astype(np.float32)
    return out[None, :]


def kernel(query: np.ndarray, keys: np.ndarray) -> np.ndarray:
    from concourse.bass_utils import run_bass_kernel_spmd

    in_maps = prepare_in_maps(query, keys)
    nc = _get_nc()
    res = run_bass_kernel_spmd(nc, in_maps, list(range(N_CORES)))
    return combine_results(res.results)
